# revision 5
# baseline (speedup 1.0000x reference)
"""Trainium2 Bass kernel for nn_ConvAttention (retrieval_knn).

Sharding: data-parallel over batch B=32 across 8 NeuronCores (4 batches/core).
All conv weights replicated (host pre-transposed + cast to bf16).

Math per batch:
  k = conv1x1(relu(conv3(keys)))            -> [80, 400]
  q = conv1x1(relu(conv1x1(relu(conv3(q)))))-> [80, 1600]
  s[t1,t2] = -0.0005*(|q_t1|^2 + |k_t2|^2 - 2 q.k)
           = 0.001*(qk - 0.5*k2[t2]) - 0.0005*q2[t1]
  The (qk - 0.5*k2) part comes out of ONE matmul via an augmented
  contraction row (lhsT row 80 = ones, rhs row 80 = -0.5*k2).
  w = exp(s)           (ACT, bias = -0.0005*q2 per partition, accum -> S1)
  u = (prior+1e-8)*w
  attn_logprob = ln(u * (1/S1))             (log_softmax + log prior, fused)
  attn = (u*maskf) / sum(u*maskf)           (masked softmax; lse cancels)
"""

import numpy as np
import ml_dtypes

B, N_MEL, N_TEXT, N_ATT, T1, T2 = 32, 80, 512, 80, 1600, 400
N_CORES = 8
B_LOC = B // N_CORES  # 4

_CACHE = {}

BF16 = ml_dtypes.bfloat16


def _build_program(opts=()):
    opts = set(opts)
    import concourse.bacc as bacc
    import concourse.tile as tile
    import concourse.mybir as mybir
    import concourse.bass as bass

    f32 = mybir.dt.float32
    bf16 = mybir.dt.bfloat16
    AF = mybir.ActivationFunctionType
    ALU = mybir.AluOpType

    nc = bacc.Bacc(None, target_bir_lowering=False)

    # ---- DRAM parameters (per-core shapes) ----
    queries_d = nc.declare_dram_parameter("queries", [B_LOC, N_MEL, T1], f32, isOutput=False)
    keys_d = nc.declare_dram_parameter("keys", [B_LOC, N_TEXT, T2], f32, isOutput=False)
    prior_d = nc.declare_dram_parameter("prior", [B_LOC, T1, T2], f32, isOutput=False)
    maskf_d = nc.declare_dram_parameter("maskf", [B_LOC, T2], f32, isOutput=False)
    # key_proj weights: conv3 512->1024 (as 4 ci-chunks x 3 taps x 8 co-chunks),
    # then conv1 1024->80 (as 8 ci-chunks)
    kw1_d = nc.declare_dram_parameter("kw1T", [4, 128, 3, 8, 128], bf16, isOutput=False)
    kb1_d = nc.declare_dram_parameter("kb1T", [128, 8], f32, isOutput=False)
    kw2_d = nc.declare_dram_parameter("kw2T", [128, 8, N_ATT], bf16, isOutput=False)
    kb2_d = nc.declare_dram_parameter("kb2r", [1, N_ATT], bf16, isOutput=False)
    # query_proj weights
    qw1_d = nc.declare_dram_parameter("qw1T", [N_MEL, 3, 160], bf16, isOutput=False)
    qb1a_d = nc.declare_dram_parameter("qb1a", [128, 1], f32, isOutput=False)
    qb1b_d = nc.declare_dram_parameter("qb1b", [32, 1], f32, isOutput=False)
    qw2a_d = nc.declare_dram_parameter("qw2a", [128, N_MEL], bf16, isOutput=False)
    qw2b_d = nc.declare_dram_parameter("qw2b", [32, N_MEL], bf16, isOutput=False)
    qb2_d = nc.declare_dram_parameter("qb2r", [N_MEL, 1], f32, isOutput=False)
    qw3_d = nc.declare_dram_parameter("qw3T", [N_MEL, N_ATT], bf16, isOutput=False)
    qb3_d = nc.declare_dram_parameter("qb3r", [1, N_ATT], bf16, isOutput=False)

    attn_d = nc.declare_dram_parameter("attn", [B_LOC, T1, T2], f32, isOutput=True)
    logp_d = nc.declare_dram_parameter("logp", [B_LOC, T1, T2], f32, isOutput=True)

    # T1 tiles: 12 x 128 + 1 x 64
    t1_tiles = []
    t0 = 0
    while t0 < T1:
        tp = min(128, T1 - t0)
        t1_tiles.append((t0, tp))
        t0 += tp

    with tile.TileContext(nc) as tc:
        from contextlib import ExitStack
        with ExitStack() as ctx:
            wp = ctx.enter_context(tc.tile_pool(name="weights", bufs=1))
            bp = ctx.enter_context(tc.tile_pool(name="perbatch", bufs=2))
            sp = ctx.enter_context(tc.tile_pool(name="score", bufs=3))
            smp = ctx.enter_context(tc.tile_pool(name="small", bufs=4))
            pp_conv = ctx.enter_context(tc.tile_pool(name="psconv", bufs=2, space="PSUM"))
            pp_k = ctx.enter_context(tc.tile_pool(name="psk", bufs=1, space="PSUM"))
            pp_s = ctx.enter_context(tc.tile_pool(name="pss", bufs=2, space="PSUM"))
            pp_col = ctx.enter_context(tc.tile_pool(name="pscol", bufs=1, space="PSUM"))
            pp_row = ctx.enter_context(tc.tile_pool(name="psrow", bufs=1, space="PSUM"))

            # ---- load weights (once) ----
            kw1_sb = []
            for kc in range(4):
                t = wp.tile([128, 3, 8, 128], bf16, tag=f"kw1_{kc}")
                nc.sync.dma_start(out=t[:], in_=kw1_d[kc])
                kw1_sb.append(t)
            kw2_sb = wp.tile([128, 8, N_ATT], bf16, tag="kw2")
            nc.sync.dma_start(out=kw2_sb[:], in_=kw2_d[:])
            kb1_sb = wp.tile([128, 8], f32, tag="kb1")
            nc.sync.dma_start(out=kb1_sb[:], in_=kb1_d[:])
            kb2_sb = wp.tile([1, N_ATT], bf16, tag="kb2")
            nc.sync.dma_start(out=kb2_sb[:], in_=kb2_d[:])
            qw1_sb = wp.tile([N_MEL, 3, 160], bf16, tag="qw1")
            nc.sync.dma_start(out=qw1_sb[:], in_=qw1_d[:])
            qb1a_sb = wp.tile([128, 1], f32, tag="qb1a")
            nc.sync.dma_start(out=qb1a_sb[:], in_=qb1a_d[:])
            qb1b_sb = wp.tile([32, 1], f32, tag="qb1b")
            nc.sync.dma_start(out=qb1b_sb[:], in_=qb1b_d[:])
            qw2a_sb = wp.tile([128, N_MEL], bf16, tag="qw2a")
            nc.sync.dma_start(out=qw2a_sb[:], in_=qw2a_d[:])
            qw2b_sb = wp.tile([32, N_MEL], bf16, tag="qw2b")
            nc.sync.dma_start(out=qw2b_sb[:], in_=qw2b_d[:])
            qb2_sb = wp.tile([N_MEL, 1], f32, tag="qb2")
            nc.sync.dma_start(out=qb2_sb[:], in_=qb2_d[:])
            qw3_sb = wp.tile([N_MEL, N_ATT], bf16, tag="qw3")
            nc.sync.dma_start(out=qw3_sb[:], in_=qw3_d[:])
            qb3_sb = wp.tile([1, N_ATT], bf16, tag="qb3")
            nc.sync.dma_start(out=qb3_sb[:], in_=qb3_d[:])

            ones_col = wp.tile([N_MEL, 1], bf16, tag="ones_col")
            nc.vector.memset(ones_col[:], 1.0)
            ones_row = wp.tile([1, T2], bf16, tag="ones_row")
            nc.vector.memset(ones_row[:], 1.0)

            n_b = 1 if "one_batch" in opts else B_LOC
            for b in range(n_b):
                # ================= key side =================
                kin = bp.tile([128, 4, T2 + 2], bf16, tag="kin")
                nc.gpsimd.memset(kin[:, :, 0:1], 0.0)
                nc.gpsimd.memset(kin[:, :, T2 + 1 : T2 + 2], 0.0)
                for kc in range(4):
                    kst = bp.tile([128, T2], f32, tag="kstage")
                    nc.sync.dma_start(out=kst[:], in_=keys_d[b, 128 * kc : 128 * (kc + 1), :])
                    nc.vector.tensor_copy(out=kin[:, kc, 1 : T2 + 1], in_=kst[:])

                k1_bf = bp.tile([128, 8, T2], bf16, tag="k1")
                for m in range(8):
                    ps = pp_conv.tile([128, T2], f32, tag="psconv")
                    for kc in range(4):
                        for d in range(3):
                            nc.tensor.matmul(
                                ps[:],
                                kw1_sb[kc][:, d, m, :],
                                kin[:, kc, d : d + T2],
                                start=(kc == 0 and d == 0),
                                stop=(kc == 3 and d == 2),
                            )
                    nc.scalar.activation(
                        out=k1_bf[:, m, :], in_=ps[:], func=AF.Relu,
                        bias=kb1_sb[:, m : m + 1], scale=1.0,
                    )

                kaug = bp.tile([97, T2], bf16, tag="kaug")
                nc.vector.memset(kaug[64:96, :], 0.0)
                psk = pp_k.tile([N_ATT, T2], f32, tag="psk")
                for m in range(8):
                    nc.tensor.matmul(psk[:], kw2_sb[:, m, :], k1_bf[:, m, :],
                                     start=(m == 0), stop=False)
                nc.tensor.matmul(psk[:], kb2_sb[:], ones_row[:, 0:T2],
                                 start=False, stop=True)
                nc.scalar.activation(out=kaug[0:N_ATT, :], in_=psk[:], func=AF.Copy,
                                     bias=0.0, scale=1.0)
                ksq = bp.tile([N_ATT, T2], bf16, tag="ksq")
                nc.vector.tensor_mul(ksq[:], kaug[0:N_ATT, :], kaug[0:N_ATT, :])
                psr = pp_row.tile([1, T2], f32, tag="psrow")
                nc.tensor.matmul(psr[:], ones_col[:], ksq[:], start=True, stop=True)
                nc.scalar.activation(out=kaug[96:97, :], in_=psr[:],
                                     func=AF.Copy, bias=0.0, scale=-0.5)

                # mask row, broadcast across partitions: 1.0 = keep, 0.0 = masked
                mrow = bp.tile([128, T2], f32, tag="mrow")
                if "no_mrow_dma" in opts:
                    nc.vector.memset(mrow[:], 1.0)
                else:
                    mb = maskf_d[b]
                    mb_bcast = bass.AP(tensor=mb.tensor, offset=mb.offset,
                                       ap=[[0, 128]] + list(mb.ap))
                    nc.gpsimd.dma_start(out=mrow[:], in_=mb_bcast)

                # ================= query side =================
                qst = bp.tile([N_MEL, T1], f32, tag="qstage")
                nc.sync.dma_start(out=qst[:], in_=queries_d[b])
                qin = bp.tile([N_MEL, T1 + 2], bf16, tag="qin")
                nc.gpsimd.memset(qin[:, 0:1], 0.0)
                nc.gpsimd.memset(qin[:, T1 + 1 : T1 + 2], 0.0)
                nc.vector.tensor_copy(out=qin[:, 1 : T1 + 1], in_=qst[:])

                q1a = bp.tile([128, T1], bf16, tag="q1a")
                q1b = bp.tile([32, T1], bf16, tag="q1b")
                q2bf = bp.tile([N_MEL, T1], bf16, tag="q2bf")
                qaug = bp.tile([97, T1], bf16, tag="qaug")
                nc.vector.memset(qaug[64:96, :], 0.0)
                for pc in range(4):  # pieces of 400 along T1
                    o = pc * T2
                    for (q1t, mp, msl, qb1t) in (
                        (q1a, 128, slice(0, 128), qb1a_sb),
                        (q1b, 32, slice(128, 160), qb1b_sb),
                    ):
                        ps = pp_conv.tile([128, T2], f32, tag="psconv")
                        for d in range(3):
                            nc.tensor.matmul(
                                ps[:mp], qw1_sb[:, d, msl], qin[:, o + d : o + d + T2],
                                start=(d == 0), stop=(d == 2),
                            )
                        nc.scalar.activation(out=q1t[:, o : o + T2], in_=ps[:mp],
                                             func=AF.Relu, bias=qb1t[:], scale=1.0)
                    ps2 = pp_k.tile([N_ATT, T2], f32, tag="psk")
                    nc.tensor.matmul(ps2[:], qw2a_sb[:], q1a[:, o : o + T2],
                                     start=True, stop=False)
                    nc.tensor.matmul(ps2[:], qw2b_sb[:], q1b[:, o : o + T2],
                                     start=False, stop=True)
                    nc.scalar.activation(out=q2bf[:, o : o + T2], in_=ps2[:],
                                         func=AF.Relu, bias=qb2_sb[:], scale=1.0)
                    ps3 = pp_k.tile([N_ATT, T2], f32, tag="psk")
                    nc.tensor.matmul(ps3[:], qw3_sb[:], q2bf[:, o : o + T2],
                                     start=True, stop=False)
                    nc.tensor.matmul(ps3[:], qb3_sb[:], ones_row[:, 0:T2],
                                     start=False, stop=True)
                    nc.scalar.activation(out=qaug[0:N_ATT, o : o + T2], in_=ps3[:],
                                         func=AF.Copy, bias=0.0, scale=1.0)
                nc.gpsimd.memset(qaug[96:97, :], 1.0)
                qsq = bp.tile([N_MEL, T1], bf16, tag="qsq")
                nc.vector.tensor_mul(qsq[:], qaug[0:N_ATT, :], qaug[0:N_ATT, :])

                # ================= score / softmax =================
                if "no_score" in opts:
                    continue
                for (o1, tp) in t1_tiles:
                    pss = pp_s.tile([128, T2], f32, tag="pss")
                    nc.tensor.matmul(pss[:tp], qaug[:, o1 : o1 + tp], kaug[:],
                                     start=True, stop=True)
                    psc = pp_col.tile([128, 1], f32, tag="pscol")
                    nc.tensor.matmul(psc[:tp], qsq[:, o1 : o1 + tp], ones_col[:],
                                     start=True, stop=True)
                    q2n = smp.tile([128, 1], f32, tag="q2n")
                    nc.vector.tensor_scalar_mul(q2n[:tp], psc[:tp], -0.0005)

                    w_t = sp.tile([128, T2], f32, tag="w")
                    S1 = smp.tile([128, 1], f32, tag="S1")
                    if "no_accum" in opts:
                        nc.scalar.activation(out=w_t[:tp], in_=pss[:tp], func=AF.Exp,
                                             bias=q2n[:tp], scale=0.001)
                        nc.vector.tensor_reduce(out=S1[:tp], in_=w_t[:tp],
                                                op=ALU.add, axis=mybir.AxisListType.X)
                    else:
                        nc.scalar.activation(out=w_t[:tp], in_=pss[:tp], func=AF.Exp,
                                             bias=q2n[:tp], scale=0.001,
                                             accum_out=S1[:tp])
                    rS1 = smp.tile([128, 1], f32, tag="rS1")
                    nc.vector.reciprocal(rS1[:tp], S1[:tp])

                    prt = sp.tile([128, T2], f32, tag="pr")
                    nc.sync.dma_start(out=prt[:tp], in_=prior_d[b, o1 : o1 + tp, :])
                    u_t = sp.tile([128, T2], f32, tag="u")
                    if "no_stt" in opts:
                        nc.vector.tensor_mul(u_t[:tp], prt[:tp], w_t[:tp])
                    else:
                        nc.vector.scalar_tensor_tensor(
                            out=u_t[:tp], in0=prt[:tp], scalar=1e-8, in1=w_t[:tp],
                            op0=ALU.add, op1=ALU.mult,
                        )
                    lp_t = sp.tile([128, T2], f32, tag="lp")
                    nc.scalar.activation(out=lp_t[:tp], in_=u_t[:tp], func=AF.Ln,
                                         bias=0.0, scale=rS1[:tp])
                    nc.sync.dma_start(out=logp_d[b, o1 : o1 + tp, :], in_=lp_t[:tp])

                    um_t = sp.tile([128, T2], f32, tag="um")
                    S2 = smp.tile([128, 1], f32, tag="S2")
                    if "no_ttr" not in opts:
                        nc.vector.scalar_tensor_tensor(
                            out=um_t[:tp], in0=u_t[:tp], scalar=1.0, in1=mrow[:tp],
                            op0=ALU.mult, op1=ALU.mult, accum_out=S2[:tp],
                        )
                    else:
                        nc.vector.tensor_mul(um_t[:tp], u_t[:tp], mrow[:tp])
                        nc.vector.tensor_reduce(out=S2[:tp], in_=um_t[:tp],
                                                op=ALU.add, axis=mybir.AxisListType.X)
                    rS2 = smp.tile([128, 1], f32, tag="rS2")
                    nc.vector.reciprocal(rS2[:tp], S2[:tp])
                    p_t = sp.tile([128, T2], f32, tag="p")
                    nc.vector.tensor_scalar_mul(p_t[:tp], um_t[:tp], rS2[:tp])
                    nc.sync.dma_start(out=attn_d[b, o1 : o1 + tp, :], in_=p_t[:tp])

    nc.compile()
    return nc


def _prep_weights(kw1, kb1, kw2, kb2, qw1, qb1, qw2, qb2, qw3, qb3):
    # kw1 [1024, 512, 3] -> [kc, ci, d, m, co]
    t = kw1.reshape(8, 128, 4, 128, 3)
    kw1T = np.ascontiguousarray(t.transpose(2, 3, 4, 0, 1)).astype(BF16)
    kb1T = np.ascontiguousarray(kb1.reshape(8, 128).T).astype(np.float32)
    # kw2 [80, 1024, 1] -> [ci128, m, co]
    t = kw2[:, :, 0].reshape(N_ATT, 8, 128)
    kw2T = np.ascontiguousarray(t.transpose(2, 1, 0)).astype(BF16)
    kb2r = np.ascontiguousarray(kb2[None, :]).astype(BF16)
    # qw1 [160, 80, 3] -> [ci, d, co]
    qw1T = np.ascontiguousarray(qw1.transpose(1, 2, 0)).astype(BF16)
    qb1a = np.ascontiguousarray(qb1[:128, None]).astype(np.float32)
    qb1b = np.ascontiguousarray(qb1[128:, None]).astype(np.float32)
    # qw2 [80, 160, 1]
    qw2a = np.ascontiguousarray(qw2[:, :128, 0].T).astype(BF16)
    qw2b = np.ascontiguousarray(qw2[:, 128:, 0].T).astype(BF16)
    qb2r = np.ascontiguousarray(qb2[:, None]).astype(np.float32)
    qw3T = np.ascontiguousarray(qw3[:, :, 0].T).astype(BF16)
    qb3r = np.ascontiguousarray(qb3[None, :]).astype(BF16)
    return dict(kw1T=kw1T, kb1T=kb1T, kw2T=kw2T, kb2r=kb2r, qw1T=qw1T,
                qb1a=qb1a, qb1b=qb1b, qw2a=qw2a, qw2b=qw2b, qb2r=qb2r,
                qw3T=qw3T, qb3r=qb3r)


def kernel(queries, keys, mask, attn_prior,
           kw1, kb1, kw2, kb2, qw1, qb1, qw2, qb2, qw3, qb3):
    from concourse.bass_utils import run_bass_kernel_spmd

    if "nc" not in _CACHE:
        _CACHE["nc"] = _build_program()
    nc = _CACHE["nc"]

    queries = np.asarray(queries, dtype=np.float32)
    keys = np.asarray(keys, dtype=np.float32)
    attn_prior = np.asarray(attn_prior, dtype=np.float32)
    maskf = (~np.asarray(mask)).astype(np.float32)  # 1.0 = keep, 0.0 = masked
    w = _prep_weights(np.asarray(kw1), np.asarray(kb1), np.asarray(kw2),
                      np.asarray(kb2), np.asarray(qw1), np.asarray(qb1),
                      np.asarray(qw2), np.asarray(qb2), np.asarray(qw3),
                      np.asarray(qb3))

    in_maps = []
    for c in range(N_CORES):
        sl = slice(B_LOC * c, B_LOC * (c + 1))
        m = {
            "queries": np.ascontiguousarray(queries[sl]),
            "keys": np.ascontiguousarray(keys[sl]),
            "prior": np.ascontiguousarray(attn_prior[sl]),
            "maskf": np.ascontiguousarray(maskf[sl]),
        }
        m.update(w)
        in_maps.append(m)

    res = run_bass_kernel_spmd(nc, in_maps, core_ids=list(range(N_CORES)),
                               **_CACHE.get("run_kwargs", {}))
    _CACHE["last_result"] = res

    attn = np.empty((B, 1, T1, T2), np.float32)
    logp = np.empty((B, 1, T1, T2), np.float32)
    for c in range(N_CORES):
        attn[B_LOC * c : B_LOC * (c + 1), 0] = res.results[c]["attn"]
        logp[B_LOC * c : B_LOC * (c + 1), 0] = res.results[c]["logp"]
    return attn, logp


# revision 8
# speedup vs baseline: 1.3283x; 1.3283x over previous
"""Trainium2 Bass kernel for nn_ConvAttention (retrieval_knn).

Sharding: data-parallel over batch B=32 across 8 NeuronCores (4 batches/core).
All conv weights replicated (host pre-transposed + cast to bf16).

Math per batch:
  k = conv1x1(relu(conv3(keys)))            -> [80, 400]
  q = conv1x1(relu(conv1x1(relu(conv3(q)))))-> [80, 1600]
  s[t1,t2] = -0.0005*(|q_t1|^2 + |k_t2|^2 - 2 q.k)
           = 0.001*(qk - 0.5*k2[t2]) - 0.0005*q2[t1]
  The (qk - 0.5*k2) part comes out of ONE matmul via an augmented
  contraction row (lhsT row 80 = ones, rhs row 80 = -0.5*k2).
  w = exp(s)           (ACT, bias = -0.0005*q2 per partition, accum -> S1)
  u = (prior+1e-8)*w
  attn_logprob = ln(u * (1/S1))             (log_softmax + log prior, fused)
  attn = (u*maskf) / sum(u*maskf)           (masked softmax; lse cancels)
"""

import numpy as np
import ml_dtypes

B, N_MEL, N_TEXT, N_ATT, T1, T2 = 32, 80, 512, 80, 1600, 400
N_CORES = 8
B_LOC = B // N_CORES  # 4

_CACHE = {}

BF16 = ml_dtypes.bfloat16


def _build_program(opts=()):
    opts = set(opts)
    import concourse.bacc as bacc
    import concourse.tile as tile
    import concourse.mybir as mybir
    import concourse.bass as bass

    f32 = mybir.dt.float32
    bf16 = mybir.dt.bfloat16
    AF = mybir.ActivationFunctionType
    ALU = mybir.AluOpType

    nc = bacc.Bacc(None, target_bir_lowering=False)

    # ---- DRAM parameters (per-core shapes) ----
    queries_d = nc.declare_dram_parameter("queries", [B_LOC, N_MEL, T1], f32, isOutput=False)
    keys_d = nc.declare_dram_parameter("keys", [B_LOC, N_TEXT, T2], f32, isOutput=False)
    prior_d = nc.declare_dram_parameter("prior", [B_LOC, T1, T2], f32, isOutput=False)
    maskf_d = nc.declare_dram_parameter("maskf", [B_LOC, T2], f32, isOutput=False)
    # key_proj weights: conv3 512->1024 (as 4 ci-chunks x 3 taps x 8 co-chunks),
    # then conv1 1024->80 (as 8 ci-chunks)
    kw1_d = nc.declare_dram_parameter("kw1T", [4, 128, 3, 8, 128], bf16, isOutput=False)
    kb1_d = nc.declare_dram_parameter("kb1T", [128, 8], f32, isOutput=False)
    kw2_d = nc.declare_dram_parameter("kw2T", [128, 8, N_ATT], bf16, isOutput=False)
    kb2_d = nc.declare_dram_parameter("kb2r", [1, N_ATT], bf16, isOutput=False)
    # query_proj weights
    qw1_d = nc.declare_dram_parameter("qw1T", [N_MEL, 3, 160], bf16, isOutput=False)
    qb1a_d = nc.declare_dram_parameter("qb1a", [128, 1], f32, isOutput=False)
    qb1b_d = nc.declare_dram_parameter("qb1b", [32, 1], f32, isOutput=False)
    qw2a_d = nc.declare_dram_parameter("qw2a", [128, N_MEL], bf16, isOutput=False)
    qw2b_d = nc.declare_dram_parameter("qw2b", [32, N_MEL], bf16, isOutput=False)
    qb2_d = nc.declare_dram_parameter("qb2r", [N_MEL, 1], f32, isOutput=False)
    qw3_d = nc.declare_dram_parameter("qw3T", [N_MEL, N_ATT], bf16, isOutput=False)
    qb3_d = nc.declare_dram_parameter("qb3r", [1, N_ATT], bf16, isOutput=False)

    attn_d = nc.declare_dram_parameter("attn", [B_LOC, T1, T2], f32, isOutput=True)
    logp_d = nc.declare_dram_parameter("logp", [B_LOC, T1, T2], f32, isOutput=True)

    # T1 tiles: 12 x 128 + 1 x 64
    t1_tiles = []
    t0 = 0
    while t0 < T1:
        tp = min(128, T1 - t0)
        t1_tiles.append((t0, tp))
        t0 += tp

    with tile.TileContext(nc) as tc:
        from contextlib import ExitStack
        with ExitStack() as ctx:
            wp = ctx.enter_context(tc.tile_pool(name="weights", bufs=1))
            bp = ctx.enter_context(tc.tile_pool(name="perbatch", bufs=2))
            sp = ctx.enter_context(tc.tile_pool(name="score", bufs=3))
            smp = ctx.enter_context(tc.tile_pool(name="small", bufs=4))
            pp_conv = ctx.enter_context(tc.tile_pool(name="psconv", bufs=2, space="PSUM"))
            pp_k = ctx.enter_context(tc.tile_pool(name="psk", bufs=1, space="PSUM"))
            pp_s = ctx.enter_context(tc.tile_pool(name="pss", bufs=2, space="PSUM"))
            pp_col = ctx.enter_context(tc.tile_pool(name="pscol", bufs=1, space="PSUM"))
            pp_row = ctx.enter_context(tc.tile_pool(name="psrow", bufs=1, space="PSUM"))

            # ---- load weights (once) ----
            kw1_sb = []
            for kc in range(4):
                t = wp.tile([128, 3, 8, 128], bf16, tag=f"kw1_{kc}")
                nc.sync.dma_start(out=t[:], in_=kw1_d[kc])
                kw1_sb.append(t)
            kw2_sb = wp.tile([128, 8, N_ATT], bf16, tag="kw2")
            nc.sync.dma_start(out=kw2_sb[:], in_=kw2_d[:])
            kb1_sb = wp.tile([128, 8], f32, tag="kb1")
            nc.sync.dma_start(out=kb1_sb[:], in_=kb1_d[:])
            kb2_sb = wp.tile([1, N_ATT], bf16, tag="kb2")
            nc.sync.dma_start(out=kb2_sb[:], in_=kb2_d[:])
            qw1_sb = wp.tile([N_MEL, 3, 160], bf16, tag="qw1")
            nc.sync.dma_start(out=qw1_sb[:], in_=qw1_d[:])
            qb1a_sb = wp.tile([128, 1], f32, tag="qb1a")
            nc.sync.dma_start(out=qb1a_sb[:], in_=qb1a_d[:])
            qb1b_sb = wp.tile([32, 1], f32, tag="qb1b")
            nc.sync.dma_start(out=qb1b_sb[:], in_=qb1b_d[:])
            qw2a_sb = wp.tile([128, N_MEL], bf16, tag="qw2a")
            nc.sync.dma_start(out=qw2a_sb[:], in_=qw2a_d[:])
            qw2b_sb = wp.tile([32, N_MEL], bf16, tag="qw2b")
            nc.sync.dma_start(out=qw2b_sb[:], in_=qw2b_d[:])
            qb2_sb = wp.tile([N_MEL, 1], f32, tag="qb2")
            nc.sync.dma_start(out=qb2_sb[:], in_=qb2_d[:])
            qw3_sb = wp.tile([N_MEL, N_ATT], bf16, tag="qw3")
            nc.sync.dma_start(out=qw3_sb[:], in_=qw3_d[:])
            qb3_sb = wp.tile([1, N_ATT], bf16, tag="qb3")
            nc.sync.dma_start(out=qb3_sb[:], in_=qb3_d[:])

            ones_col = wp.tile([N_MEL, 1], bf16, tag="ones_col")
            nc.vector.memset(ones_col[:], 1.0)
            ones_row = wp.tile([1, T2], bf16, tag="ones_row")
            nc.vector.memset(ones_row[:], 1.0)

            n_b = 1 if "one_batch" in opts else B_LOC
            for b in range(n_b):
                # ================= key side =================
                kin = bp.tile([128, 4, T2 + 2], bf16, tag="kin")
                nc.gpsimd.memset(kin[:, :, 0:1], 0.0)
                nc.gpsimd.memset(kin[:, :, T2 + 1 : T2 + 2], 0.0)
                for kc in range(4):
                    kst = bp.tile([128, T2], f32, tag="kstage")
                    nc.sync.dma_start(out=kst[:], in_=keys_d[b, 128 * kc : 128 * (kc + 1), :])
                    nc.vector.tensor_copy(out=kin[:, kc, 1 : T2 + 1], in_=kst[:])

                k1_bf = bp.tile([128, 8, T2], bf16, tag="k1")
                for m in range(8):
                    ps = pp_conv.tile([128, T2], f32, tag="psconv")
                    for kc in range(4):
                        for d in range(3):
                            nc.tensor.matmul(
                                ps[:],
                                kw1_sb[kc][:, d, m, :],
                                kin[:, kc, d : d + T2],
                                start=(kc == 0 and d == 0),
                                stop=(kc == 3 and d == 2),
                            )
                    nc.scalar.activation(
                        out=k1_bf[:, m, :], in_=ps[:], func=AF.Relu,
                        bias=kb1_sb[:, m : m + 1], scale=1.0,
                    )

                kaug = bp.tile([97, T2], bf16, tag="kaug")
                nc.vector.memset(kaug[64:96, :], 0.0)
                psk = pp_k.tile([N_ATT, T2], f32, tag="psk")
                for m in range(8):
                    nc.tensor.matmul(psk[:], kw2_sb[:, m, :], k1_bf[:, m, :],
                                     start=(m == 0), stop=False)
                nc.tensor.matmul(psk[:], kb2_sb[:], ones_row[:, 0:T2],
                                 start=False, stop=True)
                nc.scalar.activation(out=kaug[0:N_ATT, :], in_=psk[:], func=AF.Copy,
                                     bias=0.0, scale=1.0)
                ksq = bp.tile([N_ATT, T2], bf16, tag="ksq")
                nc.vector.tensor_mul(ksq[:], kaug[0:N_ATT, :], kaug[0:N_ATT, :])
                psr = pp_row.tile([1, T2], f32, tag="psrow")
                nc.tensor.matmul(psr[:], ones_col[:], ksq[:], start=True, stop=True)
                nc.scalar.activation(out=kaug[96:97, :], in_=psr[:],
                                     func=AF.Copy, bias=0.0, scale=-0.5)

                # mask row, broadcast across partitions: 1.0 = keep, 0.0 = masked
                mrow = bp.tile([128, T2], f32, tag="mrow")
                if "no_mrow_dma" in opts:
                    nc.vector.memset(mrow[:], 1.0)
                else:
                    mb = maskf_d[b]
                    mb_bcast = bass.AP(tensor=mb.tensor, offset=mb.offset,
                                       ap=[[0, 128]] + list(mb.ap))
                    nc.gpsimd.dma_start(out=mrow[:], in_=mb_bcast)

                # ================= query side =================
                qst = bp.tile([N_MEL, T1], f32, tag="qstage")
                nc.sync.dma_start(out=qst[:], in_=queries_d[b])
                qin = bp.tile([N_MEL, T1 + 2], bf16, tag="qin")
                nc.gpsimd.memset(qin[:, 0:1], 0.0)
                nc.gpsimd.memset(qin[:, T1 + 1 : T1 + 2], 0.0)
                nc.vector.tensor_copy(out=qin[:, 1 : T1 + 1], in_=qst[:])

                q1a = bp.tile([128, T1], bf16, tag="q1a")
                q1b = bp.tile([32, T1], bf16, tag="q1b")
                q2bf = bp.tile([N_MEL, T1], bf16, tag="q2bf")
                qaug = bp.tile([97, T1], bf16, tag="qaug")
                nc.vector.memset(qaug[64:96, :], 0.0)
                for pc in range(4):  # pieces of 400 along T1
                    o = pc * T2
                    for (q1t, mp, msl, qb1t) in (
                        (q1a, 128, slice(0, 128), qb1a_sb),
                        (q1b, 32, slice(128, 160), qb1b_sb),
                    ):
                        ps = pp_conv.tile([128, T2], f32, tag="psconv")
                        for d in range(3):
                            nc.tensor.matmul(
                                ps[:mp], qw1_sb[:, d, msl], qin[:, o + d : o + d + T2],
                                start=(d == 0), stop=(d == 2),
                            )
                        nc.scalar.activation(out=q1t[:, o : o + T2], in_=ps[:mp],
                                             func=AF.Relu, bias=qb1t[:], scale=1.0)
                    ps2 = pp_k.tile([N_ATT, T2], f32, tag="psk")
                    nc.tensor.matmul(ps2[:], qw2a_sb[:], q1a[:, o : o + T2],
                                     start=True, stop=False)
                    nc.tensor.matmul(ps2[:], qw2b_sb[:], q1b[:, o : o + T2],
                                     start=False, stop=True)
                    nc.scalar.activation(out=q2bf[:, o : o + T2], in_=ps2[:],
                                         func=AF.Relu, bias=qb2_sb[:], scale=1.0)
                    ps3 = pp_k.tile([N_ATT, T2], f32, tag="psk")
                    nc.tensor.matmul(ps3[:], qw3_sb[:], q2bf[:, o : o + T2],
                                     start=True, stop=False)
                    nc.tensor.matmul(ps3[:], qb3_sb[:], ones_row[:, 0:T2],
                                     start=False, stop=True)
                    nc.scalar.activation(out=qaug[0:N_ATT, o : o + T2], in_=ps3[:],
                                         func=AF.Copy, bias=0.0, scale=1.0)
                nc.gpsimd.memset(qaug[96:97, :], 1.0)
                qsq = bp.tile([N_MEL, T1], bf16, tag="qsq")
                nc.vector.tensor_mul(qsq[:], qaug[0:N_ATT, :], qaug[0:N_ATT, :])

                # ================= score / softmax =================
                if "no_score" in opts:
                    continue
                for (o1, tp) in t1_tiles:
                    pss = pp_s.tile([128, T2], f32, tag="pss")
                    nc.tensor.matmul(pss[:tp], qaug[:, o1 : o1 + tp], kaug[:],
                                     start=True, stop=True)
                    psc = pp_col.tile([128, 1], f32, tag="pscol")
                    nc.tensor.matmul(psc[:tp], qsq[:, o1 : o1 + tp], ones_col[:],
                                     start=True, stop=True)
                    q2n = smp.tile([128, 1], f32, tag="q2n")
                    nc.vector.tensor_scalar_mul(q2n[:tp], psc[:tp], -0.0005)

                    w_t = sp.tile([128, T2], f32, tag="w")
                    S1 = smp.tile([128, 1], f32, tag="S1")
                    if "no_accum" in opts:
                        nc.scalar.activation(out=w_t[:tp], in_=pss[:tp], func=AF.Exp,
                                             bias=q2n[:tp], scale=0.001)
                        nc.vector.tensor_reduce(out=S1[:tp], in_=w_t[:tp],
                                                op=ALU.add, axis=mybir.AxisListType.X)
                    else:
                        nc.scalar.activation(out=w_t[:tp], in_=pss[:tp], func=AF.Exp,
                                             bias=q2n[:tp], scale=0.001,
                                             accum_out=S1[:tp])
                    rS1 = smp.tile([128, 1], f32, tag="rS1")
                    nc.vector.reciprocal(rS1[:tp], S1[:tp])

                    prt = sp.tile([128, T2], f32, tag="pr")
                    nc.sync.dma_start(out=prt[:tp], in_=prior_d[b, o1 : o1 + tp, :])
                    u_t = sp.tile([128, T2], f32, tag="u")
                    if "no_stt" in opts:
                        nc.vector.tensor_mul(u_t[:tp], prt[:tp], w_t[:tp])
                    else:
                        nc.vector.scalar_tensor_tensor(
                            out=u_t[:tp], in0=prt[:tp], scalar=1e-8, in1=w_t[:tp],
                            op0=ALU.add, op1=ALU.mult,
                        )
                    lp_t = sp.tile([128, T2], f32, tag="lp")
                    nc.scalar.activation(out=lp_t[:tp], in_=u_t[:tp], func=AF.Ln,
                                         bias=0.0, scale=rS1[:tp])
                    nc.sync.dma_start(out=logp_d[b, o1 : o1 + tp, :], in_=lp_t[:tp])

                    um_t = sp.tile([128, T2], f32, tag="um")
                    S2 = smp.tile([128, 1], f32, tag="S2")
                    if "no_ttr" not in opts:
                        nc.vector.scalar_tensor_tensor(
                            out=um_t[:tp], in0=u_t[:tp], scalar=1.0, in1=mrow[:tp],
                            op0=ALU.mult, op1=ALU.mult, accum_out=S2[:tp],
                        )
                    else:
                        nc.vector.tensor_mul(um_t[:tp], u_t[:tp], mrow[:tp])
                        nc.vector.tensor_reduce(out=S2[:tp], in_=um_t[:tp],
                                                op=ALU.add, axis=mybir.AxisListType.X)
                    rS2 = smp.tile([128, 1], f32, tag="rS2")
                    nc.vector.reciprocal(rS2[:tp], S2[:tp])
                    p_t = sp.tile([128, T2], f32, tag="p")
                    nc.vector.tensor_scalar_mul(p_t[:tp], um_t[:tp], rS2[:tp])
                    nc.sync.dma_start(out=attn_d[b, o1 : o1 + tp, :], in_=p_t[:tp])

    # Pin ONE activation table set covering Exp/Ln/Relu/Copy, so the
    # table-load pass hoists a single ACT_TABLE_LOAD instead of thrashing
    # (~2.7us per reload, measured 80 reloads without this).
    import concourse.bacc as bacc_mod
    _orig_tabs = bacc_mod.get_activation_tables
    def _pinned_tabs(arch):
        tabs = _orig_tabs(arch)
        used = {AF.Exp, AF.Ln, AF.Relu, AF.Copy}
        out = {}
        for name, fns in tabs.items():
            if name == "natural_log_exp_and_others":
                out[name] = set(fns)
            else:
                out[name] = set(fns) - used
        return out
    bacc_mod.get_activation_tables = _pinned_tabs
    try:
        nc.compile()
    finally:
        bacc_mod.get_activation_tables = _orig_tabs
    return nc


def _prep_weights(kw1, kb1, kw2, kb2, qw1, qb1, qw2, qb2, qw3, qb3):
    # kw1 [1024, 512, 3] -> [kc, ci, d, m, co]
    t = kw1.reshape(8, 128, 4, 128, 3)
    kw1T = np.ascontiguousarray(t.transpose(2, 3, 4, 0, 1)).astype(BF16)
    kb1T = np.ascontiguousarray(kb1.reshape(8, 128).T).astype(np.float32)
    # kw2 [80, 1024, 1] -> [ci128, m, co]
    t = kw2[:, :, 0].reshape(N_ATT, 8, 128)
    kw2T = np.ascontiguousarray(t.transpose(2, 1, 0)).astype(BF16)
    kb2r = np.ascontiguousarray(kb2[None, :]).astype(BF16)
    # qw1 [160, 80, 3] -> [ci, d, co]
    qw1T = np.ascontiguousarray(qw1.transpose(1, 2, 0)).astype(BF16)
    qb1a = np.ascontiguousarray(qb1[:128, None]).astype(np.float32)
    qb1b = np.ascontiguousarray(qb1[128:, None]).astype(np.float32)
    # qw2 [80, 160, 1]
    qw2a = np.ascontiguousarray(qw2[:, :128, 0].T).astype(BF16)
    qw2b = np.ascontiguousarray(qw2[:, 128:, 0].T).astype(BF16)
    qb2r = np.ascontiguousarray(qb2[:, None]).astype(np.float32)
    qw3T = np.ascontiguousarray(qw3[:, :, 0].T).astype(BF16)
    qb3r = np.ascontiguousarray(qb3[None, :]).astype(BF16)
    return dict(kw1T=kw1T, kb1T=kb1T, kw2T=kw2T, kb2r=kb2r, qw1T=qw1T,
                qb1a=qb1a, qb1b=qb1b, qw2a=qw2a, qw2b=qw2b, qb2r=qb2r,
                qw3T=qw3T, qb3r=qb3r)


def kernel(queries, keys, mask, attn_prior,
           kw1, kb1, kw2, kb2, qw1, qb1, qw2, qb2, qw3, qb3):
    from concourse.bass_utils import run_bass_kernel_spmd

    if "nc" not in _CACHE:
        _CACHE["nc"] = _build_program()
    nc = _CACHE["nc"]

    queries = np.asarray(queries, dtype=np.float32)
    keys = np.asarray(keys, dtype=np.float32)
    attn_prior = np.asarray(attn_prior, dtype=np.float32)
    maskf = (~np.asarray(mask)).astype(np.float32)  # 1.0 = keep, 0.0 = masked
    w = _prep_weights(np.asarray(kw1), np.asarray(kb1), np.asarray(kw2),
                      np.asarray(kb2), np.asarray(qw1), np.asarray(qb1),
                      np.asarray(qw2), np.asarray(qb2), np.asarray(qw3),
                      np.asarray(qb3))

    in_maps = []
    for c in range(N_CORES):
        sl = slice(B_LOC * c, B_LOC * (c + 1))
        m = {
            "queries": np.ascontiguousarray(queries[sl]),
            "keys": np.ascontiguousarray(keys[sl]),
            "prior": np.ascontiguousarray(attn_prior[sl]),
            "maskf": np.ascontiguousarray(maskf[sl]),
        }
        m.update(w)
        in_maps.append(m)

    res = run_bass_kernel_spmd(nc, in_maps, core_ids=list(range(N_CORES)),
                               **_CACHE.get("run_kwargs", {}))
    _CACHE["last_result"] = res

    attn = np.empty((B, 1, T1, T2), np.float32)
    logp = np.empty((B, 1, T1, T2), np.float32)
    for c in range(N_CORES):
        attn[B_LOC * c : B_LOC * (c + 1), 0] = res.results[c]["attn"]
        logp[B_LOC * c : B_LOC * (c + 1), 0] = res.results[c]["logp"]
    return attn, logp


# revision 9
# speedup vs baseline: 1.4593x; 1.0986x over previous
"""Trainium2 Bass kernel for nn_ConvAttention (retrieval_knn).

Sharding: data-parallel over batch B=32 across 8 NeuronCores (4 batches/core).
All conv weights replicated (host pre-transposed + cast to bf16).

Math per batch:
  k = conv1x1(relu(conv3(keys)))            -> [80, 400]
  q = conv1x1(relu(conv1x1(relu(conv3(q)))))-> [80, 1600]
  s[t1,t2] = -0.0005*(|q_t1|^2 + |k_t2|^2 - 2 q.k)
           = 0.001*(qk - 0.5*k2[t2]) - 0.0005*q2[t1]
  The (qk - 0.5*k2) part comes out of ONE matmul via an augmented
  contraction row (lhsT row 80 = ones, rhs row 80 = -0.5*k2).
  w = exp(s)           (ACT, bias = -0.0005*q2 per partition, accum -> S1)
  u = (prior+1e-8)*w
  attn_logprob = ln(u * (1/S1))             (log_softmax + log prior, fused)
  attn = (u*maskf) / sum(u*maskf)           (masked softmax; lse cancels)
"""

import numpy as np
import ml_dtypes

B, N_MEL, N_TEXT, N_ATT, T1, T2 = 32, 80, 512, 80, 1600, 400
N_CORES = 8
B_LOC = B // N_CORES  # 4

_CACHE = {}

BF16 = ml_dtypes.bfloat16


def _build_program(opts=()):
    opts = set(opts)
    import concourse.bacc as bacc
    import concourse.tile as tile
    import concourse.mybir as mybir
    import concourse.bass as bass

    f32 = mybir.dt.float32
    bf16 = mybir.dt.bfloat16
    AF = mybir.ActivationFunctionType
    ALU = mybir.AluOpType

    nc = bacc.Bacc(None, target_bir_lowering=False)

    # ---- DRAM parameters (per-core shapes) ----
    queries_d = nc.declare_dram_parameter("queries", [B_LOC, N_MEL, T1], f32, isOutput=False)
    keys_d = nc.declare_dram_parameter("keys", [B_LOC, N_TEXT, T2], f32, isOutput=False)
    prior_d = nc.declare_dram_parameter("prior", [B_LOC, T1, T2], f32, isOutput=False)
    maskf_d = nc.declare_dram_parameter("maskf", [B_LOC, T2], f32, isOutput=False)
    # key_proj weights: conv3 512->1024 (as 4 ci-chunks x 3 taps x 8 co-chunks),
    # then conv1 1024->80 (as 8 ci-chunks)
    kw1_d = nc.declare_dram_parameter("kw1T", [4, 128, 3, 8, 128], bf16, isOutput=False)
    kb1_d = nc.declare_dram_parameter("kb1T", [128, 8], f32, isOutput=False)
    kw2_d = nc.declare_dram_parameter("kw2T", [128, 8, N_ATT], bf16, isOutput=False)
    kb2_d = nc.declare_dram_parameter("kb2r", [1, N_ATT], bf16, isOutput=False)
    # query_proj weights
    qw1_d = nc.declare_dram_parameter("qw1T", [N_MEL, 3, 160], bf16, isOutput=False)
    qb1a_d = nc.declare_dram_parameter("qb1a", [128, 1], f32, isOutput=False)
    qb1b_d = nc.declare_dram_parameter("qb1b", [32, 1], f32, isOutput=False)
    qw2a_d = nc.declare_dram_parameter("qw2a", [128, N_MEL], bf16, isOutput=False)
    qw2b_d = nc.declare_dram_parameter("qw2b", [32, N_MEL], bf16, isOutput=False)
    qb2_d = nc.declare_dram_parameter("qb2r", [N_MEL, 1], f32, isOutput=False)
    qw3_d = nc.declare_dram_parameter("qw3T", [N_MEL, N_ATT], bf16, isOutput=False)
    qb3_d = nc.declare_dram_parameter("qb3r", [1, N_ATT], bf16, isOutput=False)

    attn_d = nc.declare_dram_parameter("attn", [B_LOC, T1, T2], f32, isOutput=True)
    logp_d = nc.declare_dram_parameter("logp", [B_LOC, T1, T2], f32, isOutput=True)

    # T1 tiles: 12 x 128 + 1 x 64
    t1_tiles = []
    t0 = 0
    while t0 < T1:
        tp = min(128, T1 - t0)
        t1_tiles.append((t0, tp))
        t0 += tp

    with tile.TileContext(nc) as tc:
        from contextlib import ExitStack
        with ExitStack() as ctx:
            wp = ctx.enter_context(tc.tile_pool(name="weights", bufs=1))
            bp = ctx.enter_context(tc.tile_pool(name="perbatch", bufs=2))
            sp = ctx.enter_context(tc.tile_pool(name="score", bufs=4))
            smp = ctx.enter_context(tc.tile_pool(name="small", bufs=8))
            pp_conv = ctx.enter_context(tc.tile_pool(name="psconv", bufs=2, space="PSUM"))
            pp_k = ctx.enter_context(tc.tile_pool(name="psk", bufs=2, space="PSUM"))
            pp_s = ctx.enter_context(tc.tile_pool(name="pss", bufs=2, space="PSUM"))
            pp_col = ctx.enter_context(tc.tile_pool(name="pscol", bufs=2, space="PSUM"))

            # ---- load weights (once) ----
            kw1_sb = []
            for kc in range(4):
                t = wp.tile([128, 3, 8, 128], bf16, tag=f"kw1_{kc}")
                nc.sync.dma_start(out=t[:], in_=kw1_d[kc])
                kw1_sb.append(t)
            kw2_sb = wp.tile([128, 8, N_ATT], bf16, tag="kw2")
            nc.sync.dma_start(out=kw2_sb[:], in_=kw2_d[:])
            kb1_sb = wp.tile([128, 8], f32, tag="kb1")
            nc.sync.dma_start(out=kb1_sb[:], in_=kb1_d[:])
            kb2_sb = wp.tile([1, N_ATT], bf16, tag="kb2")
            nc.sync.dma_start(out=kb2_sb[:], in_=kb2_d[:])
            qw1_sb = wp.tile([N_MEL, 3, 160], bf16, tag="qw1")
            nc.sync.dma_start(out=qw1_sb[:], in_=qw1_d[:])
            qb1a_sb = wp.tile([128, 1], f32, tag="qb1a")
            nc.sync.dma_start(out=qb1a_sb[:], in_=qb1a_d[:])
            qb1b_sb = wp.tile([32, 1], f32, tag="qb1b")
            nc.sync.dma_start(out=qb1b_sb[:], in_=qb1b_d[:])
            qw2a_sb = wp.tile([128, N_MEL], bf16, tag="qw2a")
            nc.sync.dma_start(out=qw2a_sb[:], in_=qw2a_d[:])
            qw2b_sb = wp.tile([32, N_MEL], bf16, tag="qw2b")
            nc.sync.dma_start(out=qw2b_sb[:], in_=qw2b_d[:])
            qb2_sb = wp.tile([N_MEL, 1], f32, tag="qb2")
            nc.sync.dma_start(out=qb2_sb[:], in_=qb2_d[:])
            qw3_sb = wp.tile([N_MEL, N_ATT], bf16, tag="qw3")
            nc.sync.dma_start(out=qw3_sb[:], in_=qw3_d[:])
            qb3_sb = wp.tile([1, N_ATT], bf16, tag="qb3")
            nc.sync.dma_start(out=qb3_sb[:], in_=qb3_d[:])

            ones_col = wp.tile([N_MEL, 1], bf16, tag="ones_col")
            nc.vector.memset(ones_col[:], 1.0)
            ones_row = wp.tile([1, T2], bf16, tag="ones_row")
            nc.vector.memset(ones_row[:], 1.0)

            n_b = 1 if "one_batch" in opts else B_LOC
            for b in range(n_b):
                # ================= key side =================
                kin = bp.tile([128, 4, T2 + 2], bf16, tag="kin")
                nc.gpsimd.memset(kin[:, :, 0:1], 0.0)
                nc.gpsimd.memset(kin[:, :, T2 + 1 : T2 + 2], 0.0)
                for kc in range(4):
                    kst = bp.tile([128, T2], f32, tag="kstage")
                    nc.sync.dma_start(out=kst[:], in_=keys_d[b, 128 * kc : 128 * (kc + 1), :])
                    nc.vector.tensor_copy(out=kin[:, kc, 1 : T2 + 1], in_=kst[:])

                k1_bf = bp.tile([128, 8, T2], bf16, tag="k1")
                for m in range(8):
                    ps = pp_conv.tile([128, T2], f32, tag="psconv")
                    for kc in range(4):
                        for d in range(3):
                            nc.tensor.matmul(
                                ps[:],
                                kw1_sb[kc][:, d, m, :],
                                kin[:, kc, d : d + T2],
                                start=(kc == 0 and d == 0),
                                stop=(kc == 3 and d == 2),
                            )
                    nc.scalar.activation(
                        out=k1_bf[:, m, :], in_=ps[:], func=AF.Relu,
                        bias=kb1_sb[:, m : m + 1], scale=1.0,
                    )

                kaug = bp.tile([97, T2], bf16, tag="kaug")
                nc.vector.memset(kaug[64:96, :], 0.0)
                psk = pp_k.tile([N_ATT, T2], f32, tag="psk")
                for m in range(8):
                    nc.tensor.matmul(psk[:], kw2_sb[:, m, :], k1_bf[:, m, :],
                                     start=(m == 0), stop=False)
                nc.tensor.matmul(psk[:], kb2_sb[:], ones_row[:, 0:T2],
                                 start=False, stop=True)
                nc.scalar.activation(out=kaug[0:N_ATT, :], in_=psk[:], func=AF.Copy,
                                     bias=0.0, scale=1.0)
                ksq = bp.tile([N_ATT, T2], bf16, tag="ksq")
                nc.vector.tensor_mul(ksq[:], kaug[0:N_ATT, :], kaug[0:N_ATT, :])
                psr = pp_col.tile([1, T2], f32, tag="small")
                nc.tensor.matmul(psr[:], ones_col[:], ksq[:], start=True, stop=True)
                nc.scalar.activation(out=kaug[96:97, :], in_=psr[:],
                                     func=AF.Copy, bias=0.0, scale=-0.5)

                # mask row, broadcast across partitions: 1.0 = keep, 0.0 = masked
                mrow = bp.tile([128, T2], f32, tag="mrow")
                if "no_mrow_dma" in opts:
                    nc.vector.memset(mrow[:], 1.0)
                else:
                    mb = maskf_d[b]
                    mb_bcast = bass.AP(tensor=mb.tensor, offset=mb.offset,
                                       ap=[[0, 128]] + list(mb.ap))
                    nc.gpsimd.dma_start(out=mrow[:], in_=mb_bcast)

                # ================= query side =================
                qst = bp.tile([N_MEL, T1], f32, tag="qstage")
                nc.sync.dma_start(out=qst[:], in_=queries_d[b])
                qin = bp.tile([N_MEL, T1 + 2], bf16, tag="qin")
                nc.gpsimd.memset(qin[:, 0:1], 0.0)
                nc.gpsimd.memset(qin[:, T1 + 1 : T1 + 2], 0.0)
                nc.vector.tensor_copy(out=qin[:, 1 : T1 + 1], in_=qst[:])

                q1a = bp.tile([128, T1], bf16, tag="q1a")
                q1b = bp.tile([32, T1], bf16, tag="q1b")
                q2bf = bp.tile([N_MEL, T1], bf16, tag="q2bf")
                qaug = bp.tile([97, T1], bf16, tag="qaug")
                nc.vector.memset(qaug[64:96, :], 0.0)
                for pc in range(4):  # pieces of 400 along T1
                    o = pc * T2
                    for (q1t, mp, msl, qb1t) in (
                        (q1a, 128, slice(0, 128), qb1a_sb),
                        (q1b, 32, slice(128, 160), qb1b_sb),
                    ):
                        ps = pp_conv.tile([128, T2], f32, tag="psconv")
                        for d in range(3):
                            nc.tensor.matmul(
                                ps[:mp], qw1_sb[:, d, msl], qin[:, o + d : o + d + T2],
                                start=(d == 0), stop=(d == 2),
                            )
                        nc.scalar.activation(out=q1t[:, o : o + T2], in_=ps[:mp],
                                             func=AF.Relu, bias=qb1t[:], scale=1.0)
                    ps2 = pp_k.tile([N_ATT, T2], f32, tag="psk")
                    nc.tensor.matmul(ps2[:], qw2a_sb[:], q1a[:, o : o + T2],
                                     start=True, stop=False)
                    nc.tensor.matmul(ps2[:], qw2b_sb[:], q1b[:, o : o + T2],
                                     start=False, stop=True)
                    nc.scalar.activation(out=q2bf[:, o : o + T2], in_=ps2[:],
                                         func=AF.Relu, bias=qb2_sb[:], scale=1.0)
                    ps3 = pp_k.tile([N_ATT, T2], f32, tag="psk")
                    nc.tensor.matmul(ps3[:], qw3_sb[:], q2bf[:, o : o + T2],
                                     start=True, stop=False)
                    nc.tensor.matmul(ps3[:], qb3_sb[:], ones_row[:, 0:T2],
                                     start=False, stop=True)
                    nc.scalar.activation(out=qaug[0:N_ATT, o : o + T2], in_=ps3[:],
                                         func=AF.Copy, bias=0.0, scale=1.0)
                nc.gpsimd.memset(qaug[96:97, :], 1.0)
                qsq = bp.tile([N_MEL, T1], bf16, tag="qsq")
                nc.vector.tensor_mul(qsq[:], qaug[0:N_ATT, :], qaug[0:N_ATT, :])

                # ================= score / softmax =================
                if "no_score" in opts:
                    continue
                for (o1, tp) in t1_tiles:
                    pss = pp_s.tile([128, T2], f32, tag="pss")
                    nc.tensor.matmul(pss[:tp], qaug[:, o1 : o1 + tp], kaug[:],
                                     start=True, stop=True)
                    psc = pp_col.tile([128, 1], f32, tag="small")
                    nc.tensor.matmul(psc[:tp], qsq[:, o1 : o1 + tp], ones_col[:],
                                     start=True, stop=True)
                    q2n = smp.tile([128, 1], f32, tag="q2n")
                    nc.vector.tensor_scalar_mul(q2n[:tp], psc[:tp], -0.0005)

                    w_t = sp.tile([128, T2], f32, tag="w")
                    S1 = smp.tile([128, 1], f32, tag="S1")
                    if "no_accum" in opts:
                        nc.scalar.activation(out=w_t[:tp], in_=pss[:tp], func=AF.Exp,
                                             bias=q2n[:tp], scale=0.001)
                        nc.vector.tensor_reduce(out=S1[:tp], in_=w_t[:tp],
                                                op=ALU.add, axis=mybir.AxisListType.X)
                    else:
                        nc.scalar.activation(out=w_t[:tp], in_=pss[:tp], func=AF.Exp,
                                             bias=q2n[:tp], scale=0.001,
                                             accum_out=S1[:tp])
                    rS1 = smp.tile([128, 1], f32, tag="rS1")
                    nc.vector.reciprocal(rS1[:tp], S1[:tp])

                    prt = sp.tile([128, T2], f32, tag="pr")
                    nc.sync.dma_start(out=prt[:tp], in_=prior_d[b, o1 : o1 + tp, :])
                    u_t = sp.tile([128, T2], f32, tag="u")
                    if "no_stt" in opts:
                        nc.vector.tensor_mul(u_t[:tp], prt[:tp], w_t[:tp])
                    else:
                        nc.vector.scalar_tensor_tensor(
                            out=u_t[:tp], in0=prt[:tp], scalar=1e-8, in1=w_t[:tp],
                            op0=ALU.add, op1=ALU.mult,
                        )
                    lp_t = sp.tile([128, T2], f32, tag="lp")
                    nc.scalar.activation(out=lp_t[:tp], in_=u_t[:tp], func=AF.Ln,
                                         bias=0.0, scale=rS1[:tp])
                    nc.sync.dma_start(out=logp_d[b, o1 : o1 + tp, :], in_=lp_t[:tp])

                    um_t = sp.tile([128, T2], f32, tag="um")
                    S2 = smp.tile([128, 1], f32, tag="S2")
                    if "no_ttr" not in opts:
                        nc.vector.scalar_tensor_tensor(
                            out=um_t[:tp], in0=u_t[:tp], scalar=1.0, in1=mrow[:tp],
                            op0=ALU.mult, op1=ALU.mult, accum_out=S2[:tp],
                        )
                    else:
                        nc.vector.tensor_mul(um_t[:tp], u_t[:tp], mrow[:tp])
                        nc.vector.tensor_reduce(out=S2[:tp], in_=um_t[:tp],
                                                op=ALU.add, axis=mybir.AxisListType.X)
                    rS2 = smp.tile([128, 1], f32, tag="rS2")
                    nc.vector.reciprocal(rS2[:tp], S2[:tp])
                    p_t = sp.tile([128, T2], f32, tag="p")
                    nc.vector.tensor_scalar_mul(p_t[:tp], um_t[:tp], rS2[:tp])
                    nc.sync.dma_start(out=attn_d[b, o1 : o1 + tp, :], in_=p_t[:tp])

    # Pin ONE activation table set covering Exp/Ln/Relu/Copy, so the
    # table-load pass hoists a single ACT_TABLE_LOAD instead of thrashing
    # (~2.7us per reload, measured 80 reloads without this).
    import concourse.bacc as bacc_mod
    _orig_tabs = bacc_mod.get_activation_tables
    def _pinned_tabs(arch):
        tabs = _orig_tabs(arch)
        used = {AF.Exp, AF.Ln, AF.Relu, AF.Copy}
        out = {}
        for name, fns in tabs.items():
            if name == "natural_log_exp_and_others":
                out[name] = set(fns)
            else:
                out[name] = set(fns) - used
        return out
    bacc_mod.get_activation_tables = _pinned_tabs
    try:
        nc.compile()
    finally:
        bacc_mod.get_activation_tables = _orig_tabs
    return nc


def _prep_weights(kw1, kb1, kw2, kb2, qw1, qb1, qw2, qb2, qw3, qb3):
    # kw1 [1024, 512, 3] -> [kc, ci, d, m, co]
    t = kw1.reshape(8, 128, 4, 128, 3)
    kw1T = np.ascontiguousarray(t.transpose(2, 3, 4, 0, 1)).astype(BF16)
    kb1T = np.ascontiguousarray(kb1.reshape(8, 128).T).astype(np.float32)
    # kw2 [80, 1024, 1] -> [ci128, m, co]
    t = kw2[:, :, 0].reshape(N_ATT, 8, 128)
    kw2T = np.ascontiguousarray(t.transpose(2, 1, 0)).astype(BF16)
    kb2r = np.ascontiguousarray(kb2[None, :]).astype(BF16)
    # qw1 [160, 80, 3] -> [ci, d, co]
    qw1T = np.ascontiguousarray(qw1.transpose(1, 2, 0)).astype(BF16)
    qb1a = np.ascontiguousarray(qb1[:128, None]).astype(np.float32)
    qb1b = np.ascontiguousarray(qb1[128:, None]).astype(np.float32)
    # qw2 [80, 160, 1]
    qw2a = np.ascontiguousarray(qw2[:, :128, 0].T).astype(BF16)
    qw2b = np.ascontiguousarray(qw2[:, 128:, 0].T).astype(BF16)
    qb2r = np.ascontiguousarray(qb2[:, None]).astype(np.float32)
    qw3T = np.ascontiguousarray(qw3[:, :, 0].T).astype(BF16)
    qb3r = np.ascontiguousarray(qb3[None, :]).astype(BF16)
    return dict(kw1T=kw1T, kb1T=kb1T, kw2T=kw2T, kb2r=kb2r, qw1T=qw1T,
                qb1a=qb1a, qb1b=qb1b, qw2a=qw2a, qw2b=qw2b, qb2r=qb2r,
                qw3T=qw3T, qb3r=qb3r)


def kernel(queries, keys, mask, attn_prior,
           kw1, kb1, kw2, kb2, qw1, qb1, qw2, qb2, qw3, qb3):
    from concourse.bass_utils import run_bass_kernel_spmd

    if "nc" not in _CACHE:
        _CACHE["nc"] = _build_program()
    nc = _CACHE["nc"]

    queries = np.asarray(queries, dtype=np.float32)
    keys = np.asarray(keys, dtype=np.float32)
    attn_prior = np.asarray(attn_prior, dtype=np.float32)
    maskf = (~np.asarray(mask)).astype(np.float32)  # 1.0 = keep, 0.0 = masked
    w = _prep_weights(np.asarray(kw1), np.asarray(kb1), np.asarray(kw2),
                      np.asarray(kb2), np.asarray(qw1), np.asarray(qb1),
                      np.asarray(qw2), np.asarray(qb2), np.asarray(qw3),
                      np.asarray(qb3))

    in_maps = []
    for c in range(N_CORES):
        sl = slice(B_LOC * c, B_LOC * (c + 1))
        m = {
            "queries": np.ascontiguousarray(queries[sl]),
            "keys": np.ascontiguousarray(keys[sl]),
            "prior": np.ascontiguousarray(attn_prior[sl]),
            "maskf": np.ascontiguousarray(maskf[sl]),
        }
        m.update(w)
        in_maps.append(m)

    res = run_bass_kernel_spmd(nc, in_maps, core_ids=list(range(N_CORES)),
                               **_CACHE.get("run_kwargs", {}))
    _CACHE["last_result"] = res

    attn = np.empty((B, 1, T1, T2), np.float32)
    logp = np.empty((B, 1, T1, T2), np.float32)
    for c in range(N_CORES):
        attn[B_LOC * c : B_LOC * (c + 1), 0] = res.results[c]["attn"]
        logp[B_LOC * c : B_LOC * (c + 1), 0] = res.results[c]["logp"]
    return attn, logp
